# revision 13
# baseline (speedup 1.0000x reference)
"""Trainium2 Bass kernel for MoEAdaptorLayer (moe_routing).

Reference computation (B=512, L=50, D=768, O=300, E=8):
    gates = softmax(x @ w_gate)                          # [B,L,E]
    xw    = einsum('bli,eoi->bleo', x, expert_w)         # [B,L,E,O]
    bw    = einsum('eli,eoi->leo', expert_bias, expert_w)
    out   = einsum('ble,bleo->blo', gates, xw - bw[None])

Strategy: data-parallel over B across 8 cores (64 batches/core). Tokens are
laid out l-major per core (token = l*64 + b), so each 128-token tile covers
exactly two l values; the -bw[l] term is folded into each expert's PSUM
accumulation as one extra matmul against a constant one-hot selector (K=50).
All matmuls run in float32r (4x fp32 PE rate, ~1.5e-4 relative error).
Per tile: one contiguous 393KB DMA of pre-transposed x, 6 K-chunk matmuls
per expert (chunk-major so consecutive matmuls share the stationary operand),
gate logits ride in spare columns of expert 0's PSUM bank, softmax-normalized
gates are folded into the scalar of the vector-engine accumulation chain whose
last op writes the DMA-out tile directly.
"""

import sys

sys.path.insert(0, "/opt/trn_rl_repo")

from contextlib import ExitStack

import numpy as np

import concourse.bass as bass  # noqa: F401  (registers AP machinery)
import concourse.tile as tile
from concourse import bacc, mybir
from concourse import bass_utils

# Problem dims (hardcoded per contest contract)
B, L, D, O, E = 512, 50, 768, 300, 8
NCORES = 8
BC = B // NCORES          # 64 batches per core
TOK = BC * L              # 3200 tokens per core
P = 128                   # tokens per tile
NT = TOK // P             # 25 tiles per core
KC = D // 128             # 6 contraction chunks

F32 = mybir.dt.float32
F32R = mybir.dt.float32r

_CACHE: dict = {}


def _build_nc(reps: int = 1):
    nc = bacc.Bacc("TRN2", target_bir_lowering=False, debug=False,
                   num_devices=NCORES)

    xt_d = nc.dram_tensor("xt", [NT, P, KC, 128], F32, kind="ExternalInput").ap()
    w_d = nc.dram_tensor("w", [128, KC, E, O], F32, kind="ExternalInput").ap()
    wg_d = nc.dram_tensor("wg", [128, KC, E], F32, kind="ExternalInput").ap()
    bias_d = nc.dram_tensor("bias", [128, KC, E, L], F32, kind="ExternalInput").ap()
    out_d = nc.dram_tensor("out", [NT, P, O], F32, kind="ExternalOutput").ap()

    with tile.TileContext(nc) as tc, ExitStack() as ctx:
        const = ctx.enter_context(tc.tile_pool(name="const", bufs=1))
        stage = ctx.enter_context(tc.tile_pool(name="stage", bufs=2))
        xpool = ctx.enter_context(tc.tile_pool(name="xpool", bufs=3))
        spool = ctx.enter_context(tc.tile_pool(name="spool", bufs=3))
        opool = ctx.enter_context(tc.tile_pool(name="opool", bufs=3))
        pexp = ctx.enter_context(tc.tile_pool(name="pexp", bufs=6, space="PSUM"))
        pgate = ctx.enter_context(tc.tile_pool(name="pgate", bufs=2, space="PSUM"))

        # --- Phase 0: load + round params to f32r -------------------------
        wg_st = stage.tile([128, KC, E], F32, tag="wgst")
        nc.sync.dma_start(wg_st[:], wg_d)
        wg_sb = const.tile([128, KC, E], F32R, tag="wg")
        nc.vector.tensor_copy(wg_sb[:], wg_st[:])

        # pre-issue tile-0 x DMA so PE can start on chunk 0 ASAP
        xf0 = xpool.tile([P, KC, 128], F32, tag="xf", name="xf_pre0")
        nc.sync.dma_start(xf0[:], xt_d[0])

        # interleave w/bias chunk DMAs so chunk c lands (and can be consumed)
        # before chunk c+1; round w on ACT, bias on DVE
        w_sb, bias_sb = [], []
        for c in range(KC):
            st = stage.tile([128, E, O], F32, tag="stage", name=f"wst{c}")
            nc.sync.dma_start(st[:], w_d[:, c])
            wc = const.tile([128, E, O], F32R, tag=f"w{c}", name=f"w_sb{c}")
            nc.scalar.copy(wc[:], st[:])
            w_sb.append(wc)
            stb = stage.tile([128, E, L], F32, tag="stageb", name=f"bst{c}")
            nc.sync.dma_start(stb[:], bias_d[:, c])
            bc = const.tile([128, E, L], F32R, tag=f"b{c}", name=f"bias_sb{c}")
            nc.vector.tensor_copy(bc[:], stb[:])
            bias_sb.append(bc)

        # one-hot selector [50, NT*128]: column block t has ones at
        # (row 2t, cols 0:64) and (row 2t+1, cols 64:128), so
        # onehot[:, tP:(t+1)P].T @ negbw[:, e, :] == -bw[l(token), e, :].
        # iota value = 2t + h - l over free view [t(25), h(2), m(64)];
        # keep 1.0 where it equals 0.
        ones_st = stage.tile([L, NT * P], F32, tag="ohst")
        nc.vector.memset(ones_st[:], 1.0)
        oh_st = stage.tile([L, NT, 2, BC], F32, tag="ohst2")
        nc.gpsimd.affine_select(
            oh_st[:], ones_st[:].rearrange("l (t h m) -> l t h m", t=NT, h=2),
            pattern=[[2, NT], [1, 2], [0, BC]],
            compare_op=mybir.AluOpType.is_equal,
            fill=0.0, base=0, channel_multiplier=-1)
        onehot = const.tile([L, NT * P], F32R, tag="oh")
        nc.vector.tensor_copy(
            onehot[:], oh_st[:].rearrange("l t h m -> l (t h m)"))

        negbw = const.tile([L, E, O], F32R, tag="negbw")

        def emit_negbw():
            # negbw[l, e, o] = -sum_i expert_bias[e,l,i] * expert_w[e,o,i]
            for e in range(E):
                pbw = pexp.tile([L, O], F32, tag="pexp", name=f"pbw{e}")
                for c in range(KC):
                    nc.tensor.matmul(pbw[:], bias_sb[c][:, e, :],
                                     w_sb[c][:, e, :],
                                     start=(c == 0), stop=(c == KC - 1))
                nc.scalar.mul(negbw[:, e, :], pbw[:], -1.0)

        # --- Phase 1: token tiles ----------------------------------------
        def tile_state(rep, t, xf=None):
            if xf is None:
                xf = xpool.tile([P, KC, 128], F32, tag="xf",
                                name=f"xf{rep}_{t}")
                nc.sync.dma_start(xf[:], xt_d[t])
            xr = xpool.tile([P, KC, 128], F32R, tag="xr", name=f"xr{rep}_{t}")
            nc.scalar.copy(xr[:], xf[:])
            st = {
                "xr": xr,
                "pg": pgate.tile([P, E], F32, tag="pg", name=f"pg{rep}_{t}"),
                "gexp": spool.tile([P, E], F32, tag="gexp", name=f"gexp{rep}_{t}"),
                "gsum": spool.tile([P, 1], F32, tag="gsum", name=f"gsum{rep}_{t}"),
                "rs": spool.tile([P, 1], F32, tag="rs", name=f"rs{rep}_{t}"),
                "gn": spool.tile([P, E], F32, tag="gn", name=f"gn{rep}_{t}"),
                "acc": spool.tile([P, O], F32, tag="acc", name=f"acc{rep}_{t}"),
                "osb": opool.tile([P, O], F32, tag="osb", name=f"osb{rep}_{t}"),
            }
            return st

        def emit_group_mm(rep, t, st, g):
            xr, pg = st["xr"], st["pg"]
            pes = [pexp.tile([P, O], F32, tag="pexp",
                             name=f"pe{rep}_{t}_{g}_{j}")
                   for j in range(4)]
            for c in range(KC):
                if g == 0:
                    nc.tensor.matmul(pg[:], xr[:, c, :], wg_sb[:, c, :],
                                     start=(c == 0), stop=(c == KC - 1))
                for j in range(4):
                    e = 4 * g + j
                    nc.tensor.matmul(pes[j][:], xr[:, c, :],
                                     w_sb[c][:, e, :],
                                     start=(c == 0), stop=False,
                                     skip_group_check=True)
            return pes

        def emit_group_tail(rep, t, st, g, pes):
            gexp, gsum, rs, gn = st["gexp"], st["gsum"], st["rs"], st["gn"]
            acc, osb = st["acc"], st["osb"]
            if g == 0:
                # softmax without max-subtraction: |logits| <~ 3 here
                nc.scalar.activation(gexp[:], st["pg"][:],
                                     mybir.ActivationFunctionType.Exp,
                                     accum_out=gsum[:])
                nc.vector.reciprocal(rs[:], gsum[:])
                nc.vector.tensor_scalar_mul(gn[:], gexp[:], rs[:])
            for j in range(4):
                e = 4 * g + j
                nc.tensor.matmul(pes[j][:], onehot[:, t * P:(t + 1) * P],
                                 negbw[:, e, :],
                                 start=False, stop=True,
                                 skip_group_check=True)
                if e == 0:
                    nc.vector.tensor_scalar_mul(acc[:], pes[j][:],
                                                gn[:, 0:1])
                elif e < E - 1:
                    nc.vector.scalar_tensor_tensor(
                        acc[:], pes[j][:], gn[:, e:e + 1], acc[:],
                        op0=mybir.AluOpType.mult, op1=mybir.AluOpType.add)
                else:
                    nc.vector.scalar_tensor_tensor(
                        osb[:], pes[j][:], gn[:, e:e + 1], acc[:],
                        op0=mybir.AluOpType.mult, op1=mybir.AluOpType.add)
            if g == 1:
                nc.sync.dma_start(out_d[t], osb[:])

        # tile 0: group-0 matmuls first (can start on chunk 0 as soon as its
        # DMA lands), then negbw (needs all chunks), then tile-0 tails.
        st0 = tile_state(0, 0, xf=xf0)
        pes00 = emit_group_mm(0, 0, st0, 0)
        emit_negbw()
        emit_group_tail(0, 0, st0, 0, pes00)
        pes01 = emit_group_mm(0, 0, st0, 1)
        emit_group_tail(0, 0, st0, 1, pes01)
        for rep in range(reps):
            for t in range(NT):
                if rep == 0 and t == 0:
                    continue
                st = tile_state(rep, t)
                for g in range(2):
                    pes = emit_group_mm(rep, t, st, g)
                    emit_group_tail(rep, t, st, g, pes)

    nc.compile()
    return nc


def _prep_shared(w_gate, expert_w, expert_bias):
    w_host = np.ascontiguousarray(
        expert_w.reshape(E, O, KC, 128).transpose(3, 2, 0, 1))     # [128,6,8,300]
    wg_host = np.ascontiguousarray(
        w_gate.reshape(KC, 128, E).transpose(1, 0, 2))             # [128,6,8]
    bias_host = np.ascontiguousarray(
        expert_bias.reshape(E, L, KC, 128).transpose(3, 2, 0, 1))  # [128,6,8,50]
    return w_host, wg_host, bias_host


def kernel(x, w_gate, expert_w, expert_bias):
    x = np.asarray(x, dtype=np.float32)
    w_gate = np.asarray(w_gate, dtype=np.float32)
    expert_w = np.asarray(expert_w, dtype=np.float32)
    expert_bias = np.asarray(expert_bias, dtype=np.float32)

    if "nc" not in _CACHE:
        _CACHE["nc"] = _build_nc()
    nc = _CACHE["nc"]

    w_host, wg_host, bias_host = _prep_shared(w_gate, expert_w, expert_bias)

    in_maps = []
    for c in range(NCORES):
        xc = x[c * BC:(c + 1) * BC]                    # [64, 50, 768]
        xl = xc.transpose(1, 0, 2).reshape(TOK, D)     # l-major tokens
        xt = np.ascontiguousarray(
            xl.reshape(NT, P, KC, 128).transpose(0, 3, 2, 1))  # [25,128,6,128]
        in_maps.append({"xt": xt, "w": w_host, "wg": wg_host, "bias": bias_host})

    res = bass_utils.run_bass_kernel_spmd(nc, in_maps,
                                          core_ids=list(range(NCORES)))

    outs = []
    for c in range(NCORES):
        oc = res.results[c]["out"].reshape(L, BC, O).transpose(1, 0, 2)
        outs.append(oc)
    return np.ascontiguousarray(np.concatenate(outs, axis=0))


if __name__ == "__main__":
    rng = np.random.default_rng(0)
    inputs = {
        "x": rng.standard_normal((B, L, D), dtype=np.float32),
        "w_gate": (rng.standard_normal((D, E)) * 0.02).astype(np.float32),
        "expert_w": (rng.standard_normal((E, O, D)) * 0.02).astype(np.float32),
        "expert_bias": (rng.standard_normal((E, L, D)) * 0.02).astype(np.float32),
    }
    out = kernel(**inputs)
    print("out", out.shape, out.dtype, np.abs(out).mean())


# revision 14
# speedup vs baseline: 1.0498x; 1.0498x over previous
"""Trainium2 Bass kernel for MoEAdaptorLayer (moe_routing).

Reference computation (B=512, L=50, D=768, O=300, E=8):
    gates = softmax(x @ w_gate)                          # [B,L,E]
    xw    = einsum('bli,eoi->bleo', x, expert_w)         # [B,L,E,O]
    bw    = einsum('eli,eoi->leo', expert_bias, expert_w)
    out   = einsum('ble,bleo->blo', gates, xw - bw[None])

Strategy: data-parallel over B across 8 cores (64 batches/core). Tokens are
laid out l-major per core (token = l*64 + b), so each 128-token tile covers
exactly two l values; the -bw[l] term is folded into each expert's PSUM
accumulation as one extra matmul against a constant one-hot selector (K=50).
All matmuls run in float32r (4x fp32 PE rate, ~1.5e-4 relative error).
Per tile: one contiguous 393KB DMA of pre-transposed x, 6 K-chunk matmuls
per expert (chunk-major so consecutive matmuls share the stationary operand),
gate logits ride in spare columns of expert 0's PSUM bank, softmax-normalized
gates are folded into the scalar of the vector-engine accumulation chain whose
last op writes the DMA-out tile directly.
"""

import sys

sys.path.insert(0, "/opt/trn_rl_repo")

from contextlib import ExitStack

import numpy as np

import concourse.bass as bass  # noqa: F401  (registers AP machinery)
import concourse.tile as tile
from concourse import bacc, mybir
from concourse import bass_utils

# Problem dims (hardcoded per contest contract)
B, L, D, O, E = 512, 50, 768, 300, 8
NCORES = 8
BC = B // NCORES          # 64 batches per core
TOK = BC * L              # 3200 tokens per core
P = 128                   # tokens per tile
NT = TOK // P             # 25 tiles per core
KC = D // 128             # 6 contraction chunks

F32 = mybir.dt.float32
F32R = mybir.dt.float32r

_CACHE: dict = {}


def _build_nc(reps: int = 1):
    nc = bacc.Bacc("TRN2", target_bir_lowering=False, debug=False,
                   num_devices=NCORES)

    xt_d = nc.dram_tensor("xt", [NT, P, KC, 128], F32, kind="ExternalInput").ap()
    w_d = nc.dram_tensor("w", [128, KC, E, O], F32, kind="ExternalInput").ap()
    wg_d = nc.dram_tensor("wg", [128, KC, E], F32, kind="ExternalInput").ap()
    bias_d = nc.dram_tensor("bias", [128, KC, E, L], F32, kind="ExternalInput").ap()
    out_d = nc.dram_tensor("out", [NT, P, O], F32, kind="ExternalOutput").ap()

    with tile.TileContext(nc) as tc, ExitStack() as ctx:
        const = ctx.enter_context(tc.tile_pool(name="const", bufs=1))
        stage = ctx.enter_context(tc.tile_pool(name="stage", bufs=2))
        xpool = ctx.enter_context(tc.tile_pool(name="xpool", bufs=3))
        spool = ctx.enter_context(tc.tile_pool(name="spool", bufs=3))
        opool = ctx.enter_context(tc.tile_pool(name="opool", bufs=3))
        pexp = ctx.enter_context(tc.tile_pool(name="pexp", bufs=6, space="PSUM"))
        pgate = ctx.enter_context(tc.tile_pool(name="pgate", bufs=2, space="PSUM"))

        # --- Phase 0: load + round params to f32r -------------------------
        wg_st = stage.tile([128, KC, E], F32, tag="wgst")
        nc.sync.dma_start(wg_st[:], wg_d)
        wg_sb = const.tile([128, KC, E], F32R, tag="wg")
        nc.vector.tensor_copy(wg_sb[:], wg_st[:])

        # pre-issue tile-0 x DMA so PE can start on chunk 0 ASAP
        xf0 = xpool.tile([P, KC, 128], F32, tag="xf", name="xf_pre0")
        nc.sync.dma_start(xf0[:], xt_d[0])

        # interleave w/bias chunk DMAs so chunk c lands (and can be consumed)
        # before chunk c+1; round w on ACT, bias on DVE
        w_sb, bias_sb = [], []
        for c in range(KC):
            st = stage.tile([128, E, O], F32, tag="stage", name=f"wst{c}")
            nc.sync.dma_start(st[:], w_d[:, c])
            wc = const.tile([128, E, O], F32R, tag=f"w{c}", name=f"w_sb{c}")
            nc.scalar.copy(wc[:], st[:])
            w_sb.append(wc)
            stb = stage.tile([128, E, L], F32, tag="stageb", name=f"bst{c}")
            nc.sync.dma_start(stb[:], bias_d[:, c])
            bc = const.tile([128, E, L], F32R, tag=f"b{c}", name=f"bias_sb{c}")
            nc.vector.tensor_copy(bc[:], stb[:])
            bias_sb.append(bc)

        # one-hot selector [50, NT*128]: column block t has ones at
        # (row 2t, cols 0:64) and (row 2t+1, cols 64:128), so
        # onehot[:, tP:(t+1)P].T @ negbw[:, e, :] == -bw[l(token), e, :].
        # iota value = 2t + h - l over free view [t(25), h(2), m(64)];
        # keep 1.0 where it equals 0.
        ones_st = stage.tile([L, NT * P], F32, tag="ohst")
        nc.vector.memset(ones_st[:], 1.0)
        oh_st = stage.tile([L, NT, 2, BC], F32, tag="ohst2")
        nc.gpsimd.affine_select(
            oh_st[:], ones_st[:].rearrange("l (t h m) -> l t h m", t=NT, h=2),
            pattern=[[2, NT], [1, 2], [0, BC]],
            compare_op=mybir.AluOpType.is_equal,
            fill=0.0, base=0, channel_multiplier=-1)
        onehot = const.tile([L, NT * P], F32R, tag="oh")
        nc.vector.tensor_copy(
            onehot[:], oh_st[:].rearrange("l t h m -> l (t h m)"))

        negbw = const.tile([L, E, O], F32R, tag="negbw")

        def emit_negbw():
            # negbw[l, e, o] = -sum_i expert_bias[e,l,i] * expert_w[e,o,i]
            for e in range(E):
                pbw = pexp.tile([L, O], F32, tag="pexp", name=f"pbw{e}")
                for c in range(KC):
                    nc.tensor.matmul(pbw[:], bias_sb[c][:, e, :],
                                     w_sb[c][:, e, :],
                                     start=(c == 0), stop=(c == KC - 1))
                nc.scalar.mul(negbw[:, e, :], pbw[:], -1.0)

        # --- Phase 1: token tiles ----------------------------------------
        def tile_state(rep, t, xf=None):
            if xf is None:
                xf = xpool.tile([P, KC, 128], F32, tag="xf",
                                name=f"xf{rep}_{t}")
                nc.sync.dma_start(xf[:], xt_d[t])
            xr = xpool.tile([P, KC, 128], F32R, tag="xr", name=f"xr{rep}_{t}")
            nc.scalar.copy(xr[:], xf[:])
            st = {
                "xr": xr,
                "pg": pgate.tile([P, E], F32, tag="pg", name=f"pg{rep}_{t}"),
                "gexp": spool.tile([P, E], F32, tag="gexp", name=f"gexp{rep}_{t}"),
                "gsum": spool.tile([P, 1], F32, tag="gsum", name=f"gsum{rep}_{t}"),
                "rs": spool.tile([P, 1], F32, tag="rs", name=f"rs{rep}_{t}"),
                "gn": spool.tile([P, E], F32, tag="gn", name=f"gn{rep}_{t}"),
                "acc": spool.tile([P, O], F32, tag="acc", name=f"acc{rep}_{t}"),
                "osb": opool.tile([P, O], F32, tag="osb", name=f"osb{rep}_{t}"),
            }
            return st

        def emit_group_mm(rep, t, st, g):
            xr, pg = st["xr"], st["pg"]
            pes = [pexp.tile([P, O], F32, tag="pexp",
                             name=f"pe{rep}_{t}_{g}_{j}")
                   for j in range(4)]
            for c in range(KC):
                if g == 0:
                    nc.tensor.matmul(pg[:], xr[:, c, :], wg_sb[:, c, :],
                                     start=(c == 0), stop=(c == KC - 1))
                for j in range(4):
                    e = 4 * g + j
                    nc.tensor.matmul(pes[j][:], xr[:, c, :],
                                     w_sb[c][:, e, :],
                                     start=(c == 0), stop=False,
                                     skip_group_check=True)
            return pes

        def emit_group_tail(rep, t, st, g, pes):
            gexp, gsum, rs, gn = st["gexp"], st["gsum"], st["rs"], st["gn"]
            acc, osb = st["acc"], st["osb"]
            if g == 0:
                # softmax without max-subtraction: |logits| <~ 3 here
                nc.scalar.activation(gexp[:], st["pg"][:],
                                     mybir.ActivationFunctionType.Exp,
                                     accum_out=gsum[:])
                nc.vector.reciprocal(rs[:], gsum[:])
                nc.vector.tensor_scalar_mul(gn[:], gexp[:], rs[:])
            for j in range(4):
                e = 4 * g + j
                nc.tensor.matmul(pes[j][:], onehot[:, t * P:(t + 1) * P],
                                 negbw[:, e, :],
                                 start=False, stop=True,
                                 skip_group_check=True)
                if e == 0:
                    nc.vector.tensor_scalar_mul(acc[:], pes[j][:],
                                                gn[:, 0:1])
                elif e < E - 1:
                    nc.vector.scalar_tensor_tensor(
                        acc[:], pes[j][:], gn[:, e:e + 1], acc[:],
                        op0=mybir.AluOpType.mult, op1=mybir.AluOpType.add)
                else:
                    nc.vector.scalar_tensor_tensor(
                        osb[:], pes[j][:], gn[:, e:e + 1], acc[:],
                        op0=mybir.AluOpType.mult, op1=mybir.AluOpType.add)
            if g == 1:
                nc.sync.dma_start(out_d[t], osb[:])

        # consolidated start: PE begins once params are resident and then
        # runs without idle gaps (avoids HAM clock-gate re-throttle).
        emit_negbw()
        for rep in range(reps):
            for t in range(NT):
                st = tile_state(rep, t, xf=xf0 if (rep == 0 and t == 0) else None)
                for g in range(2):
                    pes = emit_group_mm(rep, t, st, g)
                    emit_group_tail(rep, t, st, g, pes)

    nc.compile()
    return nc


def _prep_shared(w_gate, expert_w, expert_bias):
    w_host = np.ascontiguousarray(
        expert_w.reshape(E, O, KC, 128).transpose(3, 2, 0, 1))     # [128,6,8,300]
    wg_host = np.ascontiguousarray(
        w_gate.reshape(KC, 128, E).transpose(1, 0, 2))             # [128,6,8]
    bias_host = np.ascontiguousarray(
        expert_bias.reshape(E, L, KC, 128).transpose(3, 2, 0, 1))  # [128,6,8,50]
    return w_host, wg_host, bias_host


def kernel(x, w_gate, expert_w, expert_bias):
    x = np.asarray(x, dtype=np.float32)
    w_gate = np.asarray(w_gate, dtype=np.float32)
    expert_w = np.asarray(expert_w, dtype=np.float32)
    expert_bias = np.asarray(expert_bias, dtype=np.float32)

    if "nc" not in _CACHE:
        _CACHE["nc"] = _build_nc()
    nc = _CACHE["nc"]

    w_host, wg_host, bias_host = _prep_shared(w_gate, expert_w, expert_bias)

    in_maps = []
    for c in range(NCORES):
        xc = x[c * BC:(c + 1) * BC]                    # [64, 50, 768]
        xl = xc.transpose(1, 0, 2).reshape(TOK, D)     # l-major tokens
        xt = np.ascontiguousarray(
            xl.reshape(NT, P, KC, 128).transpose(0, 3, 2, 1))  # [25,128,6,128]
        in_maps.append({"xt": xt, "w": w_host, "wg": wg_host, "bias": bias_host})

    res = bass_utils.run_bass_kernel_spmd(nc, in_maps,
                                          core_ids=list(range(NCORES)))

    outs = []
    for c in range(NCORES):
        oc = res.results[c]["out"].reshape(L, BC, O).transpose(1, 0, 2)
        outs.append(oc)
    return np.ascontiguousarray(np.concatenate(outs, axis=0))


if __name__ == "__main__":
    rng = np.random.default_rng(0)
    inputs = {
        "x": rng.standard_normal((B, L, D), dtype=np.float32),
        "w_gate": (rng.standard_normal((D, E)) * 0.02).astype(np.float32),
        "expert_w": (rng.standard_normal((E, O, D)) * 0.02).astype(np.float32),
        "expert_bias": (rng.standard_normal((E, L, D)) * 0.02).astype(np.float32),
    }
    out = kernel(**inputs)
    print("out", out.shape, out.dtype, np.abs(out).mean())


# revision 15
# speedup vs baseline: 1.1926x; 1.1360x over previous
"""Trainium2 Bass kernel for MoEAdaptorLayer (moe_routing).

Reference computation (B=512, L=50, D=768, O=300, E=8):
    gates = softmax(x @ w_gate)                          # [B,L,E]
    xw    = einsum('bli,eoi->bleo', x, expert_w)         # [B,L,E,O]
    bw    = einsum('eli,eoi->leo', expert_bias, expert_w)
    out   = einsum('ble,bleo->blo', gates, xw - bw[None])

Strategy: data-parallel over B across 8 cores (64 batches/core). Tokens are
laid out l-major per core (token = l*64 + b), so each 128-token tile covers
exactly two l values; the -bw[l] term is folded into each expert's PSUM
accumulation as one extra matmul against a constant one-hot selector (K=50).
All matmuls run in float32r (4x fp32 PE rate, ~1.5e-4 relative error).
Per tile: one contiguous 393KB DMA of pre-transposed x, 6 K-chunk matmuls
per expert (chunk-major so consecutive matmuls share the stationary operand),
gate logits ride in spare columns of expert 0's PSUM bank, softmax-normalized
gates are folded into the scalar of the vector-engine accumulation chain whose
last op writes the DMA-out tile directly.
"""

import sys

sys.path.insert(0, "/opt/trn_rl_repo")

from contextlib import ExitStack

import numpy as np

import concourse.bass as bass  # noqa: F401  (registers AP machinery)
import concourse.tile as tile
from concourse import bacc, mybir
from concourse import bass_utils

# Problem dims (hardcoded per contest contract)
B, L, D, O, E = 512, 50, 768, 300, 8
NCORES = 8
BC = B // NCORES          # 64 batches per core
TOK = BC * L              # 3200 tokens per core
P = 128                   # tokens per tile
NT = TOK // P             # 25 tiles per core
KC = D // 128             # 6 contraction chunks

F32 = mybir.dt.float32
F32R = mybir.dt.float32r

_CACHE: dict = {}


def _build_nc(reps: int = 1, mmdt=F32R):
    nc = bacc.Bacc("TRN2", target_bir_lowering=False, debug=False,
                   num_devices=NCORES)

    xt_d = nc.dram_tensor("xt", [NT, P, KC, 128], F32, kind="ExternalInput").ap()
    w_d = nc.dram_tensor("w", [128, KC, E, O], F32, kind="ExternalInput").ap()
    wg_d = nc.dram_tensor("wg", [128, KC, E], F32, kind="ExternalInput").ap()
    bias_d = nc.dram_tensor("bias", [128, KC, E, L], F32, kind="ExternalInput").ap()
    out_d = nc.dram_tensor("out", [NT, P, O], F32, kind="ExternalOutput").ap()

    with tile.TileContext(nc) as tc, ExitStack() as ctx:
        const = ctx.enter_context(tc.tile_pool(name="const", bufs=1))
        stage = ctx.enter_context(tc.tile_pool(name="stage", bufs=2))
        xpool = ctx.enter_context(tc.tile_pool(name="xpool", bufs=3))
        spool = ctx.enter_context(tc.tile_pool(name="spool", bufs=3))
        opool = ctx.enter_context(tc.tile_pool(name="opool", bufs=3))
        pexp = ctx.enter_context(tc.tile_pool(name="pexp", bufs=6, space="PSUM"))
        pgate = ctx.enter_context(tc.tile_pool(name="pgate", bufs=2, space="PSUM"))

        # --- Phase 0: load + round params to f32r -------------------------
        wg_st = stage.tile([128, KC, E], F32, tag="wgst")
        nc.sync.dma_start(wg_st[:], wg_d)
        wg_sb = const.tile([128, KC, E], mmdt, tag="wg")
        nc.vector.tensor_copy(wg_sb[:], wg_st[:])

        # pre-issue tile-0 x DMA so PE can start on chunk 0 ASAP
        xf0 = xpool.tile([P, KC, 128], F32, tag="xf", name="xf_pre0")
        nc.sync.dma_start(xf0[:], xt_d[0])

        # interleave w/bias chunk DMAs so chunk c lands (and can be consumed)
        # before chunk c+1; round w on ACT, bias on DVE
        w_sb, bias_sb = [], []
        for c in range(KC):
            st = stage.tile([128, E, O], F32, tag="stage", name=f"wst{c}")
            nc.sync.dma_start(st[:], w_d[:, c])
            wc = const.tile([128, E, O], mmdt, tag=f"w{c}", name=f"w_sb{c}")
            nc.scalar.copy(wc[:], st[:])
            w_sb.append(wc)
            stb = stage.tile([128, E, L], F32, tag="stageb", name=f"bst{c}")
            nc.sync.dma_start(stb[:], bias_d[:, c])
            bc = const.tile([128, E, L], mmdt, tag=f"b{c}", name=f"bias_sb{c}")
            nc.vector.tensor_copy(bc[:], stb[:])
            bias_sb.append(bc)

        # one-hot selector [50, NT*128]: column block t has ones at
        # (row 2t, cols 0:64) and (row 2t+1, cols 64:128), so
        # onehot[:, tP:(t+1)P].T @ negbw[:, e, :] == -bw[l(token), e, :].
        # iota value = 2t + h - l over free view [t(25), h(2), m(64)];
        # keep 1.0 where it equals 0.
        ones_st = stage.tile([L, NT * P], F32, tag="ohst")
        nc.vector.memset(ones_st[:], 1.0)
        oh_st = stage.tile([L, NT, 2, BC], F32, tag="ohst2")
        nc.gpsimd.affine_select(
            oh_st[:], ones_st[:].rearrange("l (t h m) -> l t h m", t=NT, h=2),
            pattern=[[2, NT], [1, 2], [0, BC]],
            compare_op=mybir.AluOpType.is_equal,
            fill=0.0, base=0, channel_multiplier=-1)
        onehot = const.tile([L, NT * P], mmdt, tag="oh")
        nc.vector.tensor_copy(
            onehot[:], oh_st[:].rearrange("l t h m -> l (t h m)"))

        negbw = const.tile([L, E, O], mmdt, tag="negbw")

        def emit_negbw():
            # negbw[l, e, o] = -sum_i expert_bias[e,l,i] * expert_w[e,o,i]
            for e in range(E):
                pbw = pexp.tile([L, O], F32, tag="pexp", name=f"pbw{e}")
                for c in range(KC):
                    nc.tensor.matmul(pbw[:], bias_sb[c][:, e, :],
                                     w_sb[c][:, e, :],
                                     start=(c == 0), stop=(c == KC - 1))
                nc.scalar.mul(negbw[:, e, :], pbw[:], -1.0)

        # --- Phase 1: token tiles ----------------------------------------
        def tile_state(rep, t, xf=None):
            if xf is None:
                xf = xpool.tile([P, KC, 128], F32, tag="xf",
                                name=f"xf{rep}_{t}")
                nc.sync.dma_start(xf[:], xt_d[t])
            xr = xpool.tile([P, KC, 128], mmdt, tag="xr", name=f"xr{rep}_{t}")
            nc.scalar.copy(xr[:], xf[:])
            st = {
                "xr": xr,
                "pg": pgate.tile([P, E], F32, tag="pg", name=f"pg{rep}_{t}"),
                "gexp": spool.tile([P, E], F32, tag="gexp", name=f"gexp{rep}_{t}"),
                "gsum": spool.tile([P, 1], F32, tag="gsum", name=f"gsum{rep}_{t}"),
                "rs": spool.tile([P, 1], F32, tag="rs", name=f"rs{rep}_{t}"),
                "gn": spool.tile([P, E], F32, tag="gn", name=f"gn{rep}_{t}"),
                "acc": spool.tile([P, O], F32, tag="acc", name=f"acc{rep}_{t}"),
                "osb": opool.tile([P, O], F32, tag="osb", name=f"osb{rep}_{t}"),
            }
            return st

        def emit_group_mm(rep, t, st, g):
            xr, pg = st["xr"], st["pg"]
            pes = [pexp.tile([P, O], F32, tag="pexp",
                             name=f"pe{rep}_{t}_{g}_{j}")
                   for j in range(4)]
            for c in range(KC):
                if g == 0:
                    nc.tensor.matmul(pg[:], xr[:, c, :], wg_sb[:, c, :],
                                     start=(c == 0), stop=(c == KC - 1))
                for j in range(4):
                    e = 4 * g + j
                    nc.tensor.matmul(pes[j][:], xr[:, c, :],
                                     w_sb[c][:, e, :],
                                     start=(c == 0), stop=False,
                                     skip_group_check=True)
            return pes

        def emit_group_tail(rep, t, st, g, pes):
            gexp, gsum, rs, gn = st["gexp"], st["gsum"], st["rs"], st["gn"]
            acc, osb = st["acc"], st["osb"]
            if g == 0:
                # softmax without max-subtraction: |logits| <~ 3 here
                nc.scalar.activation(gexp[:], st["pg"][:],
                                     mybir.ActivationFunctionType.Exp,
                                     accum_out=gsum[:])
                nc.vector.reciprocal(rs[:], gsum[:])
                nc.vector.tensor_scalar_mul(gn[:], gexp[:], rs[:])
            for j in range(4):
                e = 4 * g + j
                nc.tensor.matmul(pes[j][:], onehot[:, t * P:(t + 1) * P],
                                 negbw[:, e, :],
                                 start=False, stop=True,
                                 skip_group_check=True)
                if e == 0:
                    nc.vector.tensor_scalar_mul(acc[:], pes[j][:],
                                                gn[:, 0:1])
                elif e < E - 1:
                    nc.vector.scalar_tensor_tensor(
                        acc[:], pes[j][:], gn[:, e:e + 1], acc[:],
                        op0=mybir.AluOpType.mult, op1=mybir.AluOpType.add)
                else:
                    nc.vector.scalar_tensor_tensor(
                        osb[:], pes[j][:], gn[:, e:e + 1], acc[:],
                        op0=mybir.AluOpType.mult, op1=mybir.AluOpType.add)
            if g == 1:
                nc.sync.dma_start(out_d[t], osb[:])

        # consolidated start: PE begins once params are resident and then
        # runs without idle gaps (avoids HAM clock-gate re-throttle).
        emit_negbw()
        for rep in range(reps):
            for t in range(NT):
                st = tile_state(rep, t, xf=xf0 if (rep == 0 and t == 0) else None)
                for g in range(2):
                    pes = emit_group_mm(rep, t, st, g)
                    emit_group_tail(rep, t, st, g, pes)

    nc.compile()
    return nc


def _prep_shared(w_gate, expert_w, expert_bias):
    w_host = np.ascontiguousarray(
        expert_w.reshape(E, O, KC, 128).transpose(3, 2, 0, 1))     # [128,6,8,300]
    wg_host = np.ascontiguousarray(
        w_gate.reshape(KC, 128, E).transpose(1, 0, 2))             # [128,6,8]
    bias_host = np.ascontiguousarray(
        expert_bias.reshape(E, L, KC, 128).transpose(3, 2, 0, 1))  # [128,6,8,50]
    return w_host, wg_host, bias_host


def kernel(x, w_gate, expert_w, expert_bias):
    x = np.asarray(x, dtype=np.float32)
    w_gate = np.asarray(w_gate, dtype=np.float32)
    expert_w = np.asarray(expert_w, dtype=np.float32)
    expert_bias = np.asarray(expert_bias, dtype=np.float32)

    if "nc" not in _CACHE:
        _CACHE["nc"] = _build_nc()
    nc = _CACHE["nc"]

    w_host, wg_host, bias_host = _prep_shared(w_gate, expert_w, expert_bias)

    in_maps = []
    for c in range(NCORES):
        xc = x[c * BC:(c + 1) * BC]                    # [64, 50, 768]
        xl = xc.transpose(1, 0, 2).reshape(TOK, D)     # l-major tokens
        xt = np.ascontiguousarray(
            xl.reshape(NT, P, KC, 128).transpose(0, 3, 2, 1))  # [25,128,6,128]
        in_maps.append({"xt": xt, "w": w_host, "wg": wg_host, "bias": bias_host})

    res = bass_utils.run_bass_kernel_spmd(nc, in_maps,
                                          core_ids=list(range(NCORES)))

    outs = []
    for c in range(NCORES):
        oc = res.results[c]["out"].reshape(L, BC, O).transpose(1, 0, 2)
        outs.append(oc)
    return np.ascontiguousarray(np.concatenate(outs, axis=0))


if __name__ == "__main__":
    rng = np.random.default_rng(0)
    inputs = {
        "x": rng.standard_normal((B, L, D), dtype=np.float32),
        "w_gate": (rng.standard_normal((D, E)) * 0.02).astype(np.float32),
        "expert_w": (rng.standard_normal((E, O, D)) * 0.02).astype(np.float32),
        "expert_bias": (rng.standard_normal((E, L, D)) * 0.02).astype(np.float32),
    }
    out = kernel(**inputs)
    print("out", out.shape, out.dtype, np.abs(out).mean())


# revision 17
# speedup vs baseline: 1.3432x; 1.1263x over previous
"""Trainium2 Bass kernel for MoEAdaptorLayer (moe_routing).

Reference computation (B=512, L=50, D=768, O=300, E=8):
    gates = softmax(x @ w_gate)                          # [B,L,E]
    xw    = einsum('bli,eoi->bleo', x, expert_w)         # [B,L,E,O]
    bw    = einsum('eli,eoi->leo', expert_bias, expert_w)
    out   = einsum('ble,bleo->blo', gates, xw - bw[None])

Strategy: data-parallel over B across 8 cores (64 batches/core); no
collectives. Tokens are laid out l-major per core (token = l*64 + b), so each
128-token tile covers exactly two l values. Matmul operands are rounded to
fp16 on device (fp32 PSUM accumulation; ~3e-4 relative error) — fp16 streams
at full PE rate where fp32 runs at 1/4. Per 128-token tile:
  - one contiguous 393KB DMA of host-pre-transposed x, rounded to fp16 on ACT;
  - 6 K-chunk matmuls per expert, chunk-major so the stationary operand is
    shared; gate-logit columns are packed ahead of expert 0's weights so one
    matmul stream computes [gates | expert0] into one PSUM bank;
  - softmax on ACT/DVE; normalized gates are transposed via the PE so the
    gate-weighted bias correction  -sum_e g_e bw[l,e,:]  is two tiny K=8
    matmuls into PSUM (one per l-half);
  - the 8 expert outputs are folded as acc = sum_e g_e * P_e + corr by a
    vector-engine scalar_tensor_tensor chain whose first op reads the corr
    PSUM bank and whose last op writes the DMA-out tile directly.
"""

import sys

sys.path.insert(0, "/opt/trn_rl_repo")

from contextlib import ExitStack

import numpy as np

import concourse.bass as bass  # noqa: F401
import concourse.tile as tile
from concourse import bacc, mybir
from concourse import bass_utils
from concourse.masks import make_identity

B, L, D, O, E = 512, 50, 768, 300, 8
NCORES = 8
BC = B // NCORES          # 64 batches per core
TOK = BC * L              # 3200 tokens per core
P = 128                   # tokens per tile
NT = TOK // P             # 25 tiles per core
KC = D // 128             # 6 contraction chunks
WCOL = E + E * O          # packed w row: [gate(8) | e0(300) | ... | e7(300)]

F32 = mybir.dt.float32
FP16 = mybir.dt.float16

_CACHE: dict = {}


def _build_nc(reps: int = 1, mmdt=FP16):
    nc = bacc.Bacc("TRN2", target_bir_lowering=False, debug=False,
                   num_devices=NCORES)

    xt_d = nc.dram_tensor("xt", [NT, P, KC, 128], F32, kind="ExternalInput").ap()
    w_d = nc.dram_tensor("w", [128, KC, WCOL], F32, kind="ExternalInput").ap()
    bias_d = nc.dram_tensor("bias", [128, KC, E, L], F32, kind="ExternalInput").ap()
    out_d = nc.dram_tensor("out", [NT, P, O], F32, kind="ExternalOutput").ap()

    with tile.TileContext(nc) as tc, ExitStack() as ctx:
        const = ctx.enter_context(tc.tile_pool(name="const", bufs=1))
        stage = ctx.enter_context(tc.tile_pool(name="stage", bufs=2))
        xpool = ctx.enter_context(tc.tile_pool(name="xpool", bufs=3))
        spool = ctx.enter_context(tc.tile_pool(name="spool", bufs=3))
        opool = ctx.enter_context(tc.tile_pool(name="opool", bufs=3))
        dpool = ctx.enter_context(tc.tile_pool(name="dram", bufs=1, space="DRAM"))
        pexp = ctx.enter_context(tc.tile_pool(name="pexp", bufs=7, space="PSUM"))
        pcor = ctx.enter_context(tc.tile_pool(name="pcor", bufs=1, space="PSUM"))

        # --- Phase 0: params ---------------------------------------------
        # pre-issue tile-0 x DMA so it's resident when the PE ramps up
        xf0 = xpool.tile([P, KC, 128], F32, tag="xf", name="xf_pre0")
        nc.sync.dma_start(xf0[:], xt_d[0])

        w_sb, bias_sb = [], []
        for c in range(KC):
            st = stage.tile([128, WCOL], F32, tag="stage", name=f"wst{c}")
            nc.sync.dma_start(st[:], w_d[:, c])
            wc = const.tile([128, WCOL], mmdt, tag=f"w{c}", name=f"w_sb{c}")
            nc.scalar.copy(wc[:], st[:])
            w_sb.append(wc)
            stb = stage.tile([128, E, L], F32, tag="stageb", name=f"bst{c}")
            nc.sync.dma_start(stb[:], bias_d[:, c])
            bc = const.tile([128, E, L], mmdt, tag=f"b{c}", name=f"bias_sb{c}")
            nc.vector.tensor_copy(bc[:], stb[:])
            bias_sb.append(bc)

        ident = const.tile([128, 128], F32, tag="ident")
        make_identity(nc, ident[:])

        # negbw[e, l*O+o] = -sum_i expert_bias[e,l,i] * expert_w[e,o,i],
        # e on partitions (rhs layout for the tiny corr matmuls).
        # Computed [l, o] per expert in PSUM, negated+rounded to fp16,
        # relaid out through a DRAM bounce.
        negbwT = const.tile([E, L * O], mmdt, tag="negbwT")
        scratch = dpool.tile([E, L, O], mmdt, tag="nbscratch")

        def emit_negbw():
            for e in range(E):
                pbw = pexp.tile([L, O], F32, tag="pexp", name=f"pbw{e}")
                for c in range(KC):
                    nc.tensor.matmul(pbw[:], bias_sb[c][:, e, :],
                                     w_sb[c][:, E + e * O:E + (e + 1) * O],
                                     start=(c == 0), stop=(c == KC - 1))
                nbst = stage.tile([L, O], mmdt, tag="nbst", name=f"nbst{e}")
                nc.scalar.mul(nbst[:], pbw[:], -1.0)
                nc.sync.dma_start(scratch[e], nbst[:])
            nc.sync.dma_start(negbwT[:],
                              scratch[:].rearrange("e l o -> e (l o)"))

        # --- Phase 1: token tiles ----------------------------------------
        def tile_state(rep, t, xf=None):
            if xf is None:
                xf = xpool.tile([P, KC, 128], F32, tag="xf",
                                name=f"xf{rep}_{t}")
                nc.sync.dma_start(xf[:], xt_d[t])
            xr = xpool.tile([P, KC, 128], mmdt, tag="xr", name=f"xr{rep}_{t}")
            nc.scalar.copy(xr[:], xf[:])
            return {
                "xr": xr,
                "gexp": spool.tile([P, E], F32, tag="gexp", name=f"gexp{rep}_{t}"),
                "gsum": spool.tile([P, 1], F32, tag="gsum", name=f"gsum{rep}_{t}"),
                "rs": spool.tile([P, 1], F32, tag="rs", name=f"rs{rep}_{t}"),
                "gn": spool.tile([P, E], F32, tag="gn", name=f"gn{rep}_{t}"),
                "gts": spool.tile([E, P], mmdt, tag="gts", name=f"gts{rep}_{t}"),
                "acc": spool.tile([P, O], F32, tag="acc", name=f"acc{rep}_{t}"),
                "osb": opool.tile([P, O], F32, tag="osb", name=f"osb{rep}_{t}"),
            }

        def emit_group_mm(rep, t, st, g):
            xr = st["xr"]
            pes = []
            for j in range(4):
                e = 4 * g + j
                wid = O + E if e == 0 else O
                pes.append(pexp.tile([P, wid], F32, tag="pexp",
                                     name=f"pe{rep}_{t}_{g}_{j}"))
            for c in range(KC):
                for j in range(4):
                    e = 4 * g + j
                    lo = 0 if e == 0 else E + e * O
                    nc.tensor.matmul(pes[j][:], xr[:, c, :],
                                     w_sb[c][:, lo:lo + pes[j].shape[-1]],
                                     start=(c == 0), stop=(c == KC - 1))
            return pes

        def emit_group_tail(rep, t, st, g, pes, ptr):
            gexp, gsum, rs, gn = st["gexp"], st["gsum"], st["rs"], st["gn"]
            acc, osb = st["acc"], st["osb"]
            if g == 0:
                # softmax without max-subtraction (|logits| <~ 3 here);
                # gate logits live in cols 0:8 of expert-0's PSUM bank
                nc.scalar.activation(gexp[:], pes[0][:, 0:E],
                                     mybir.ActivationFunctionType.Exp,
                                     accum_out=gsum[:])
                nc.vector.reciprocal(rs[:], gsum[:])
                nc.vector.tensor_scalar_mul(gn[:], gexp[:], rs[:])
                # gate-weighted bias correction:
                #   corr[m, :] = -sum_e gn[m,e] * bw[l(m), e, :]
                # via gn^T (PE transpose) and two K=8 matmuls, one per l-half
                nc.tensor.transpose(ptr[0:E, 0:P], gn[:], ident[:])
                nc.vector.tensor_copy(st["gts"][:], ptr[0:E, 0:P])
                for h in range(2):
                    lt = 2 * t + h
                    nc.tensor.matmul(ptr[h * BC:(h + 1) * BC, :],
                                     st["gts"][:, h * BC:(h + 1) * BC],
                                     negbwT[:, lt * O:(lt + 1) * O],
                                     start=True, stop=True,
                                     skip_group_check=True)
            if g == 0:
                # corr PSUM -> acc (ScalarE), so the DVE chain reads only one
                # PSUM operand per op
                nc.scalar.copy(acc[:], ptr[:])
            for j in range(4):
                e = 4 * g + j
                pj = pes[j][:, E:E + O] if e == 0 else pes[j][:]
                if e == 0:
                    nc.vector.scalar_tensor_tensor(
                        acc[:], pj, gn[:, 0:1], acc[:],
                        op0=mybir.AluOpType.mult, op1=mybir.AluOpType.add)
                elif e < E - 1:
                    nc.vector.scalar_tensor_tensor(
                        acc[:], pj, gn[:, e:e + 1], acc[:],
                        op0=mybir.AluOpType.mult, op1=mybir.AluOpType.add)
                else:
                    nc.vector.scalar_tensor_tensor(
                        osb[:], pj, gn[:, e:e + 1], acc[:],
                        op0=mybir.AluOpType.mult, op1=mybir.AluOpType.add)
            if g == 1:
                nc.sync.dma_start(out_d[t], osb[:])

        emit_negbw()
        for rep in range(reps):
            for t in range(NT):
                st = tile_state(rep, t, xf=xf0 if (rep == 0 and t == 0) else None)
                ptr = pcor.tile([P, O], F32, tag="pcor", name=f"pc{rep}_{t}")
                for g in range(2):
                    pes = emit_group_mm(rep, t, st, g)
                    emit_group_tail(rep, t, st, g, pes, ptr)

    nc.compile()
    return nc


def _prep_shared(w_gate, expert_w, expert_bias):
    # packed per-chunk weight rows: [gate(8) | expert0(300) | ... | expert7(300)]
    wg_c = w_gate.reshape(KC, 128, E).transpose(1, 0, 2)            # [128,6,8]
    we_c = expert_w.reshape(E, O, KC, 128).transpose(3, 2, 0, 1)    # [128,6,8,300]
    w_host = np.ascontiguousarray(np.concatenate(
        [wg_c, we_c.reshape(128, KC, E * O)], axis=2))              # [128,6,2408]
    bias_host = np.ascontiguousarray(
        expert_bias.reshape(E, L, KC, 128).transpose(3, 2, 0, 1))   # [128,6,8,50]
    return w_host, bias_host


def kernel(x, w_gate, expert_w, expert_bias):
    x = np.asarray(x, dtype=np.float32)
    w_gate = np.asarray(w_gate, dtype=np.float32)
    expert_w = np.asarray(expert_w, dtype=np.float32)
    expert_bias = np.asarray(expert_bias, dtype=np.float32)

    if "nc" not in _CACHE:
        _CACHE["nc"] = _build_nc()
    nc = _CACHE["nc"]

    w_host, bias_host = _prep_shared(w_gate, expert_w, expert_bias)

    in_maps = []
    for c in range(NCORES):
        xc = x[c * BC:(c + 1) * BC]                    # [64, 50, 768]
        xl = xc.transpose(1, 0, 2).reshape(TOK, D)     # l-major tokens
        xt = np.ascontiguousarray(
            xl.reshape(NT, P, KC, 128).transpose(0, 3, 2, 1))  # [25,128,6,128]
        in_maps.append({"xt": xt, "w": w_host, "bias": bias_host})

    res = bass_utils.run_bass_kernel_spmd(nc, in_maps,
                                          core_ids=list(range(NCORES)))

    outs = []
    for c in range(NCORES):
        oc = res.results[c]["out"].reshape(L, BC, O).transpose(1, 0, 2)
        outs.append(oc)
    return np.ascontiguousarray(np.concatenate(outs, axis=0))


if __name__ == "__main__":
    rng = np.random.default_rng(0)
    inputs = {
        "x": rng.standard_normal((B, L, D), dtype=np.float32),
        "w_gate": (rng.standard_normal((D, E)) * 0.02).astype(np.float32),
        "expert_w": (rng.standard_normal((E, O, D)) * 0.02).astype(np.float32),
        "expert_bias": (rng.standard_normal((E, L, D)) * 0.02).astype(np.float32),
    }
    out = kernel(**inputs)
    print("out", out.shape, out.dtype, np.abs(out).mean())
